# revision 16
# baseline (speedup 1.0000x reference)
"""GaussianMLP sampling kernel for 8 trn2 NeuronCores (pure data parallel).

reference:
    h      = relu(x @ W_emb + b_emb)        x:[B,128] W_emb:[128,256]
    mean   = h @ W_mean + b_mean            W_mean:[256,128]
    logvar = h @ W_logvar + b_logvar        W_logvar:[256,128]
    z      = mean + exp(0.5*logvar) * eps
    returns (z, mean, logvar)

v2 design (transposed dataflow, all-bf16 I/O):
  - Host stages xT/epsT as bf16 [128, rows] per core, weights as bf16,
    so the kernel never transposes on-chip and HBM traffic is halved.
  - All compute keeps the feature dim on partitions:
      hT[dh, r]   = We_chunk.T @ xT          (2 matmuls, K=d_in=128)
      meanT[do,r] = Wm_chunk.T @ hT_chunk    (2 matmuls accum, K=dh)
      lvT [do,r]  = Wl_chunk.T @ hT_chunk    (2 matmuls accum)
    so L2 biases are per-partition -> folded into ACT/DVE/Pool bias ops,
    no PSUM bias seeding and no PE transposes at all.
  - Outputs written bf16 transposed [128, rows]; host casts/transposes back.
  - Epilogue spread across ACT (relu, exp), Pool (relu, mean+bias),
    DVE (lv+bias, std*eps, z-add).
  - DMA in 4096-row super-tiles (1 MiB per stream) on the sync HWDGE queue.
"""

import sys

sys.path.insert(0, "/opt/trn_rl_repo")

import numpy as np
import ml_dtypes

from contextlib import ExitStack

from concourse import bacc, bass, mybir, tile
from concourse.alu_op_type import AluOpType
from concourse.bass_utils import run_bass_kernel_spmd

N_CORES = 8
B = 524288
D_IN = 128
D_H = 256
D_OUT = 128
ROWS_PER_CORE = B // N_CORES  # 65536
R_DMA = 4096  # rows per DMA super-tile
R_SUB = 512  # rows per compute subtile
N_T = ROWS_PER_CORE // R_DMA  # 16
N_U = R_DMA // R_SUB  # 8

F32 = mybir.dt.float32
BF16 = mybir.dt.bfloat16
NP_BF16 = ml_dtypes.bfloat16

AF = mybir.ActivationFunctionType


def build_bass(rows_per_core=ROWS_PER_CORE):
    nc = bacc.Bacc("TRN2", target_bir_lowering=False, debug=False)
    n_t = rows_per_core // R_DMA

    xT_ext = nc.declare_dram_parameter("xT", [D_IN, rows_per_core], BF16, isOutput=False)
    epsT_ext = nc.declare_dram_parameter(
        "epsT", [D_OUT, rows_per_core], BF16, isOutput=False
    )
    We_ext = nc.declare_dram_parameter("W_emb", [D_IN, D_H], BF16, isOutput=False)
    be_ext = nc.declare_dram_parameter("b_emb", [D_H], F32, isOutput=False)
    Wm_ext = nc.declare_dram_parameter("W_mean", [D_H, D_OUT], BF16, isOutput=False)
    bm_ext = nc.declare_dram_parameter("b_mean", [D_OUT], F32, isOutput=False)
    Wl_ext = nc.declare_dram_parameter("W_logvar", [D_H, D_OUT], BF16, isOutput=False)
    bl_ext = nc.declare_dram_parameter("b_logvar", [D_OUT], F32, isOutput=False)
    blh_ext = nc.declare_dram_parameter("b_logvar_half", [D_OUT], F32, isOutput=False)
    zT_ext = nc.declare_dram_parameter("zT", [D_OUT, rows_per_core], BF16, isOutput=True)
    mT_ext = nc.declare_dram_parameter(
        "meanT", [D_OUT, rows_per_core], BF16, isOutput=True
    )
    lT_ext = nc.declare_dram_parameter(
        "logvarT", [D_OUT, rows_per_core], BF16, isOutput=True
    )

    with tile.TileContext(nc) as tc, ExitStack() as ctx:
        const = ctx.enter_context(tc.tile_pool(name="const", bufs=1))
        xin = ctx.enter_context(tc.tile_pool(name="xin", bufs=3))
        epool = ctx.enter_context(tc.tile_pool(name="eps", bufs=3))
        hpool = ctx.enter_context(tc.tile_pool(name="hT", bufs=3))
        spool = ctx.enter_context(tc.tile_pool(name="small", bufs=3))
        stg = ctx.enter_context(tc.tile_pool(name="stg", bufs=3))
        psH = ctx.enter_context(tc.tile_pool(name="psH", bufs=2, space="PSUM"))
        psO = ctx.enter_context(tc.tile_pool(name="psO", bufs=2, space="PSUM"))

        # --- constants / weights (loaded once) ---
        We_sb = const.tile([128, D_H], BF16)
        nc.sync.dma_start(We_sb[:], We_ext[:])
        Wm_sb = const.tile([128, 2, D_OUT], BF16)
        Wl_sb = const.tile([128, 2, D_OUT], BF16)
        nc.sync.dma_start(Wm_sb[:], Wm_ext.rearrange("(c p) d -> p c d", p=128))
        nc.sync.dma_start(Wl_sb[:], Wl_ext.rearrange("(c p) d -> p c d", p=128))
        be_sb = const.tile([128, 2], F32)
        nc.sync.dma_start(be_sb[:], be_ext.rearrange("(c p) -> p c", p=128))
        bm_sb = const.tile([128, 1], F32)
        nc.sync.dma_start(bm_sb[:], bm_ext.rearrange("(p o) -> p o", o=1))
        bl_sb = const.tile([128, 1], F32)
        nc.sync.dma_start(bl_sb[:], bl_ext.rearrange("(p o) -> p o", o=1))
        blh_sb = const.tile([128, 1], F32)
        nc.sync.dma_start(blh_sb[:], blh_ext.rearrange("(p o) -> p o", o=1))

        def emit_l1(t, u, x_sb):
            """L1 for subtile u of DMA tile t: hT psum + relu to SBUF bf16."""
            xs = x_sb[:, u * R_SUB : (u + 1) * R_SUB]
            h_ps0 = psH.tile([128, R_SUB], F32, tag="h0")
            h_ps1 = psH.tile([128, R_SUB], F32, tag="h1")
            nc.tensor.matmul(h_ps0[:], We_sb[:, 0:128], xs, start=True, stop=True)
            nc.tensor.matmul(h_ps1[:], We_sb[:, 128:256], xs, start=True, stop=True)
            h_sb0 = hpool.tile([128, R_SUB], BF16, tag="h0")
            h_sb1 = hpool.tile([128, R_SUB], BF16, tag="h1")
            nc.scalar.activation(h_sb0[:], h_ps0[:], AF.Relu, bias=be_sb[:, 0:1])
            nc.scalar.activation(h_sb1[:], h_ps1[:], AF.Relu, bias=be_sb[:, 1:2])
            return h_sb0, h_sb1

        def emit_l2(t, u, h_sb0, h_sb1, eps_sb, z_st, m_st, l_st):
            """L2 + epilogue for subtile u: writes bf16 slices into staging."""
            sl = slice(u * R_SUB, (u + 1) * R_SUB)
            m_ps = psO.tile([128, R_SUB], F32, tag="m")
            l_ps = psO.tile([128, R_SUB], F32, tag="l")
            nc.tensor.matmul(m_ps[:], Wm_sb[:, 0, :], h_sb0[:], start=True, stop=False)
            nc.tensor.matmul(m_ps[:], Wm_sb[:, 1, :], h_sb1[:], start=False, stop=True)
            nc.tensor.matmul(l_ps[:], Wl_sb[:, 0, :], h_sb0[:], start=True, stop=False)
            nc.tensor.matmul(l_ps[:], Wl_sb[:, 1, :], h_sb1[:], start=False, stop=True)

            # logvar out = lv_ps + bl   (DVE);  std = exp(0.5*lv_ps + 0.5*bl) (ACT)
            nc.vector.tensor_scalar(
                l_st[:, sl], l_ps[:], bl_sb[:, 0:1], None, AluOpType.add
            )
            std_sb = spool.tile([128, R_SUB], BF16, tag="std")
            nc.scalar.activation(std_sb[:], l_ps[:], AF.Exp, bias=blh_sb[:, 0:1], scale=0.5)
            # mean out = m_ps + bm  (DVE)
            nc.vector.tensor_scalar(
                m_st[:, sl], m_ps[:], bm_sb[:, 0:1], None, AluOpType.add
            )
            # z = mean + std*eps  (se alternates DVE/GpSimd; z-add on GpSimd)
            se_sb = spool.tile([128, R_SUB], BF16, tag="se")
            se_eng = nc.vector if (u % 2 == 0) else nc.gpsimd
            se_eng.tensor_tensor(
                se_sb[:], std_sb[:], eps_sb[:, sl], AluOpType.mult
            )
            nc.gpsimd.tensor_tensor(
                z_st[:, sl], m_st[:, sl], se_sb[:], AluOpType.add
            )

        for t in range(n_t):
            c0, c1 = t * R_DMA, (t + 1) * R_DMA
            x_sb = xin.tile([128, R_DMA], BF16, tag="x")
            nc.sync.dma_start(x_sb[:], xT_ext[:, c0:c1])
            eps_sb = epool.tile([128, R_DMA], BF16, tag="e")
            nc.sync.dma_start(eps_sb[:], epsT_ext[:, c0:c1])
            z_st = stg.tile([128, R_DMA], BF16, tag="z")
            m_st = stg.tile([128, R_DMA], BF16, tag="m")
            l_st = stg.tile([128, R_DMA], BF16, tag="l")

            # software-pipelined: emit L1(u) ahead of L2(u-1) so the PE
            # stream never waits on the relu of the subtile it just made
            prev = None
            for u in range(N_U):
                h0, h1 = emit_l1(t, u, x_sb)
                if prev is not None:
                    emit_l2(t, u - 1, *prev, eps_sb, z_st, m_st, l_st)
                prev = (h0, h1)
            emit_l2(t, N_U - 1, *prev, eps_sb, z_st, m_st, l_st)

            nc.sync.dma_start(zT_ext[:, c0:c1], z_st[:])
            nc.sync.dma_start(mT_ext[:, c0:c1], m_st[:])
            nc.sync.dma_start(lT_ext[:, c0:c1], l_st[:])

    nc.finalize()
    return nc


_NC_CACHE = None


def _get_nc():
    global _NC_CACHE
    if _NC_CACHE is None:
        _NC_CACHE = build_bass()
    return _NC_CACHE


def _run(inputs, trace=False, **kw):
    nc = _get_nc()
    f32 = np.float32
    x = np.asarray(inputs["x"], dtype=f32)
    eps = np.asarray(inputs["eps"], dtype=f32)
    weights = {
        "W_emb": np.ascontiguousarray(np.asarray(inputs["W_emb"], f32).astype(NP_BF16)),
        "W_mean": np.ascontiguousarray(np.asarray(inputs["W_mean"], f32).astype(NP_BF16)),
        "W_logvar": np.ascontiguousarray(
            np.asarray(inputs["W_logvar"], f32).astype(NP_BF16)
        ),
        "b_emb": np.ascontiguousarray(np.asarray(inputs["b_emb"], f32)),
        "b_mean": np.ascontiguousarray(np.asarray(inputs["b_mean"], f32)),
        "b_logvar": np.ascontiguousarray(np.asarray(inputs["b_logvar"], f32)),
        "b_logvar_half": np.ascontiguousarray(
            0.5 * np.asarray(inputs["b_logvar"], f32)
        ),
    }
    in_maps = []
    for c in range(N_CORES):
        sl = slice(c * ROWS_PER_CORE, (c + 1) * ROWS_PER_CORE)
        in_maps.append(
            {
                "xT": np.ascontiguousarray(x[sl].T.astype(NP_BF16)),
                "epsT": np.ascontiguousarray(eps[sl].T.astype(NP_BF16)),
                **weights,
            }
        )
    res = run_bass_kernel_spmd(nc, in_maps, list(range(N_CORES)), trace=trace, **kw)

    def gather(name):
        out = np.empty((B, D_OUT), dtype=f32)
        for c in range(N_CORES):
            sl = slice(c * ROWS_PER_CORE, (c + 1) * ROWS_PER_CORE)
            out[sl] = np.asarray(res.results[c][name], dtype=f32).T
        return out

    return (gather("zT"), gather("meanT"), gather("logvarT")), res


def kernel(**inputs):
    out, _ = _run(inputs, trace=False)
    return out


if __name__ == "__main__":
    rng = np.random.default_rng(0)
    demo = {
        "x": rng.standard_normal((B, D_IN), dtype=np.float32),
        "eps": rng.standard_normal((B, D_OUT), dtype=np.float32),
        "W_emb": rng.standard_normal((D_IN, D_H), dtype=np.float32) * 0.088,
        "b_emb": rng.standard_normal((D_H,), dtype=np.float32) * 0.05,
        "W_mean": rng.standard_normal((D_H, D_OUT), dtype=np.float32) * 0.06,
        "b_mean": rng.standard_normal((D_OUT,), dtype=np.float32) * 0.03,
        "W_logvar": rng.standard_normal((D_H, D_OUT), dtype=np.float32) * 0.06,
        "b_logvar": rng.standard_normal((D_OUT,), dtype=np.float32) * 0.03,
    }
    z, m, l = kernel(**demo)
    print("shapes", z.shape, m.shape, l.shape, z.dtype)
